# revision 1
# baseline (speedup 1.0000x reference)
"""Distributed causal multi-head attention layer on 8 TRN2 NeuronCores.

Problem (hardcoded): x [4, 2048, 1024] f32, qkv_w [1024, 3072], qkv_b [3072],
proj_w [1024, 1024], proj_b [1024]; 16 heads, head_dim 64, causal softmax.

Sharding: core i handles batch b = i//2 and head group g = i%2 (8 heads,
512 channels). Each core computes x[b] @ qkv slice -> causal attention for
its heads -> partial projection [2048, 1024]. Host sums the two partials
per batch and adds proj_b. No collectives.

Per-core layout (bf16 on the TensorEngine, f32 accumulation):
  xT  [C=1024, T=2048]  transposed on the host; stored as three wide SBUF
                        tiles (quarter0, quarter1, half-hi) so each loads
                        with ONE 3D-AP DMA (the ~0.8us/descriptor issue
                        cost on the queue engines dominated startup)
  QT,KT [512, T]        d-on-partitions; head h lives at partition offset
                        64*(h%2) of tile h//2 -> even/odd head score matmuls
                        auto-derive PE tile_position (0,0)/(64,0) and run
                        row-tiled *concurrently* when issued back to back
  V_aug [T, 8*128]      per head: ones column ++ 63 zeros ++ V_h. The ones
                        column FIRST puts the softmax denominators at PSUM
                        partition 0 so reciprocal_approx_fast (which drops
                        input partition offsets) reads them directly - no
                        staging copy on the normalize critical path; V in
                        the upper half so the O^T rows start at the
                        32-aligned partition 64 (PSUM access rule)
  S^T pair [128, 2x512] one PSUM tile holds both heads of a pair; a single
                        ScalarE exp (3D AP) covers both (fewer ACTIVATEs)
  P' = exp(S^T/8)       no max subtraction (|S| <~ 3 for this distribution)
  O^T[128, i] += V_aug^T @ P'  per head, K=128 accumulation over j tiles,
                        lagging the scores by 4-5 j-tiles; row 0 = sums.
                        J-tiles run in pairs (scores t,t+1 back to back,
                        then both lagged OT pairs) to halve the exposed
                        scores<->OT stationary-transition drains
  normalize             reciprocal_approx_fast + gpsimd partition_broadcast
  Y = OTn^T @ W2        proj partial -> DMA out bf16 (host sums in f32)

Scheduling: one software-pipelined stream paced by attention j-tiles.
12 dummy warm-up matmuls run during the input-DMA window so the PE HAM
clock-gate is 8/8 when real work lands. A 6-chunk upfront block (QK chunk-0
stripe-0 + V tiles 0-3) unblocks attention pair 0 at ~13us; every other
dense chunk is a deadline-tagged filler popped into the attention stream
just-in-time (deadline = first consuming j-tile) with a 1-per-2-j-tile
background pace and minimum spacing so the 2-slot filler PSUM never stalls
the PE on its DVE drain. Pairs 0,1 run i-block-ascending; pairs 2,3
interleave by i-block so projection spreads through the tail.
"""

import sys

for _p in ("/opt/trn_rl_repo",):
    if _p not in sys.path:
        sys.path.insert(0, _p)

import numpy as np
import ml_dtypes

import concourse.bass as bass
import concourse.tile as tile
from concourse import bacc, mybir
from concourse.bass_utils import run_bass_kernel_spmd

BF16NP = ml_dtypes.bfloat16
F32 = mybir.dt.float32
BF16 = mybir.dt.bfloat16

B, T, C = 4, 2048, 1024
H, DH = 16, 64
N_CORES = 8
HL = 8           # heads per core
DL = HL * DH     # 512 channels per core
CCN = C // 128   # 8 contraction chunks
DCN = DL // 128  # 4 d-chunks of the local 512 channels
NT = T // 128    # 16 t-tiles
IBN = T // 512   # 4 i-blocks for attention

_cached_nc = None
DEBUG_DUMPS = False


def _build():
    global _cached_nc
    if _cached_nc is not None:
        return _cached_nc

    nc = bacc.Bacc("TRN2", target_bir_lowering=False, debug=False,
                   num_devices=N_CORES)
    dbg = {}
    if DEBUG_DUMPS:
        dbg["qt0"] = nc.dram_tensor("dbg_qt0", [128, T], BF16,
                                    kind="ExternalOutput").ap()
        dbg["otn0"] = nc.dram_tensor("dbg_otn0", [128, T], BF16,
                                     kind="ExternalOutput").ap()

    # inputs pre-tiled on the host: each is one contiguous 2D HBM read
    xtq0a_ap = nc.dram_tensor("xtq0a", [128, CCN * 256], BF16,
                              kind="ExternalInput").ap()
    xtq0b_ap = nc.dram_tensor("xtq0b", [128, CCN * 256], BF16,
                              kind="ExternalInput").ap()
    xtq1_ap = nc.dram_tensor("xtq1", [128, CCN * 512], BF16,
                             kind="ExternalInput").ap()
    xthi_ap = nc.dram_tensor("xthi", [128, CCN * 1024], BF16,
                             kind="ExternalInput").ap()
    wqd0_ap = nc.dram_tensor("wqd0", [128, CCN * 128], BF16,
                             kind="ExternalInput").ap()
    wqre_ap = nc.dram_tensor("wqre", [128, CCN * 384], BF16,
                             kind="ExternalInput").ap()
    wkd0_ap = nc.dram_tensor("wkd0", [128, CCN * 128], BF16,
                             kind="ExternalInput").ap()
    wkre_ap = nc.dram_tensor("wkre", [128, CCN * 384], BF16,
                             kind="ExternalInput").ap()
    wv_ap = nc.dram_tensor("wv", [128, CCN * 512], BF16,
                           kind="ExternalInput").ap()
    w2_ap = nc.dram_tensor("w2", [128, DCN * 1024], BF16,
                           kind="ExternalInput").ap()
    qb_ap = nc.dram_tensor("qb", [DL], F32, kind="ExternalInput").ap()
    kb_ap = nc.dram_tensor("kb", [DL], F32, kind="ExternalInput").ap()
    vb_ap = nc.dram_tensor("vb", [1, DL], F32, kind="ExternalInput").ap()
    m0_ap = nc.dram_tensor("m0", [128, 256], BF16, kind="ExternalInput").ap()
    out_ap = nc.dram_tensor("out", [T, C], BF16, kind="ExternalOutput").ap()

    Act = mybir.ActivationFunctionType

    with tile.TileContext(nc) as tc:
        with (
            tc.tile_pool(name="persist", bufs=1) as pp,
            tc.tile_pool(name="big_psum", bufs=2, space="PSUM") as bp,
            tc.tile_pool(name="fill_psum", bufs=2, space="PSUM") as fp,
            tc.tile_pool(name="ot_psum", bufs=2, space="PSUM") as op,
            tc.tile_pool(name="work", bufs=6) as wp,
            tc.tile_pool(name="outbuf", bufs=3) as yp,
        ):
            # ---- persistent SBUF tensors ----
            # x^T in three wide tiles, one DMA each: quarters 0,1 (cols
            # 0:512, 512:1024 of every chunk) and the high half
            xt_q0a = pp.tile([128, CCN * 256], BF16, tag="xtq0a",
                             name="xt_q0a")
            xt_q0b = pp.tile([128, CCN * 256], BF16, tag="xtq0b",
                             name="xt_q0b")
            xt_q1 = pp.tile([128, CCN * 512], BF16, tag="xtq1", name="xt_q1")
            xt_hi = pp.tile([128, CCN * 1024], BF16, tag="xthi", name="xt_hi")

            def xt_sl(cc, lo, hi):
                """AP for x^T chunk cc, columns [lo, hi)."""
                if hi <= 256:
                    return xt_q0a[:, cc * 256 + lo:cc * 256 + hi]
                if hi <= 512:
                    return xt_q0b[:, cc * 256 + lo - 256:cc * 256 + hi - 256]
                if hi <= 1024:
                    return xt_q1[:, cc * 512 + lo - 512:cc * 512 + hi - 512]
                return xt_hi[:, cc * 1024 + lo - 1024:cc * 1024 + hi - 1024]

            # qkv weights: d-chunk 0 slice separate from the rest so the
            # first QK chains depend on a 256KB DMA, not a 1MB one
            wq_d0 = pp.tile([128, CCN * 128], BF16, tag="wqd0", name="wq_d0")
            wq_re = pp.tile([128, CCN * 384], BF16, tag="wqre", name="wq_re")
            wk_d0 = pp.tile([128, CCN * 128], BF16, tag="wkd0", name="wk_d0")
            wk_re = pp.tile([128, CCN * 384], BF16, tag="wkre", name="wk_re")
            wv_sb = pp.tile([128, CCN * 512], BF16, tag="wv", name="wv_sb")
            w2_sb = pp.tile([128, DCN * 1024], BF16, tag="w2", name="w2_sb")

            def w_sl(which, cc, dc):
                if which == "q":
                    return (wq_d0[:, cc * 128:(cc + 1) * 128] if dc == 0
                            else wq_re[:, cc * 384 + (dc - 1) * 128:
                                       cc * 384 + dc * 128])
                return (wk_d0[:, cc * 128:(cc + 1) * 128] if dc == 0
                        else wk_re[:, cc * 384 + (dc - 1) * 128:
                                   cc * 384 + dc * 128])

            qt = [pp.tile([128, T], BF16, tag=f"qt{i}", name=f"qt{i}")
                  for i in range(DCN)]
            kt = [pp.tile([128, T], BF16, tag=f"kt{i}", name=f"kt{i}")
                  for i in range(DCN)]
            otn = [pp.tile([128, T], BF16, tag=f"otn{i}", name=f"otn{i}")
                   for i in range(DCN)]
            vaug = [pp.tile([128, HL * 128], BF16, tag=f"va{i}", name=f"va{i}")
                    for i in range(NT)]
            qb_sb = pp.tile([128, DCN], F32, tag="qb", name="qb_sb")
            kb_sb = pp.tile([128, DCN], F32, tag="kb", name="kb_sb")
            vb_sb = pp.tile([1, DL], F32, tag="vb", name="vb_sb")
            vb_bc = pp.tile([128, DL], F32, tag="vbb", name="vb_bc")
            m0_sb = pp.tile([128, 256], BF16, tag="m0", name="m0_sb")
            warm_sb = pp.tile([128, 512], BF16, tag="warm", name="warm_sb")

            # ---- input DMAs: ~512KB pieces alternating between the two
            # HWDGE queues (sync, scalar), priority order. Note a "half"
            # of a chunk-major tile is chunks 0-3 / 4-7, and every matmul
            # chain reads ALL chunks, so both halves of a tensor are
            # equally critical - they just ride different queues.
            def _half(t, ap, h):
                n = t.shape[-1]
                return (t[:, h * n // 2:(h + 1) * n // 2],
                        ap[:, h * n // 2:(h + 1) * n // 2])

            for eng, (dst, src) in (
                (nc.sync, (wq_d0[:], wqd0_ap[:])),
                (nc.scalar, (wk_d0[:], wkd0_ap[:])),
                (nc.sync, (xt_q0a[:], xtq0a_ap[:])),
                (nc.scalar, (xt_q0b[:], xtq0b_ap[:])),
                (nc.sync, _half(wv_sb, wv_ap, 0)),
                (nc.scalar, _half(wv_sb, wv_ap, 1)),
                (nc.sync, _half(xt_q1, xtq1_ap, 0)),
                (nc.scalar, _half(xt_q1, xtq1_ap, 1)),
                (nc.sync, _half(xt_hi, xthi_ap, 0)),
                (nc.scalar, _half(xt_hi, xthi_ap, 1)),
                (nc.sync, _half(wq_re, wqre_ap, 0)),
                (nc.scalar, _half(wq_re, wqre_ap, 1)),
                (nc.sync, _half(wk_re, wkre_ap, 0)),
                (nc.scalar, _half(wk_re, wkre_ap, 1)),
            ):
                eng.dma_start(out=dst, in_=src)
            # small/late tensors on the gpsimd SWDGE queue
            nc.gpsimd.dma_start(out=m0_sb[:], in_=m0_ap[:])
            nc.gpsimd.dma_start(out=vb_sb[:], in_=vb_ap[:])
            nc.gpsimd.dma_start(out=qb_sb[:],
                                in_=qb_ap.rearrange("(a p) -> p a", p=128))
            nc.gpsimd.dma_start(out=kb_sb[:],
                                in_=kb_ap.rearrange("(a p) -> p a", p=128))
            nc.gpsimd.partition_broadcast(vb_bc[:], vb_sb[:])
            nc.gpsimd.dma_start(out=w2_sb[:], in_=w2_ap[:])

            # ---- HAM warm-up: dummy accumulating matmuls fill the input
            # DMA window so the PE clock-gate is 8/8 when real work lands
            nc.vector.memset(warm_sb[:], 0.0)
            ps_warm = fp.tile([128, 512], F32, tag="fill", name="ps_warm")
            NWARM = 14
            for i in range(NWARM):
                nc.tensor.matmul(ps_warm[:], lhsT=warm_sb[:, 0:128],
                                 rhs=warm_sb[:],
                                 start=(i == 0), stop=(i == NWARM - 1))

            def v_tile(tt, pool, tagname):
                """V projection t-tile: natural layout [t=128, d=512]."""
                t0 = tt * 128
                ps_v = pool.tile([128, DL], F32, tag=tagname,
                                 name=f"psv{tt}")
                for cc in range(CCN):
                    nc.tensor.matmul(ps_v[:], lhsT=xt_sl(cc, t0, t0 + 128),
                                     rhs=wv_sb[:, cc * 512:(cc + 1) * 512],
                                     start=(cc == 0), stop=(cc == CCN - 1))
                # ones first (softmax sums at PSUM partition 0), V in the
                # upper half so O^T lands at partitions 64-127 (PSUM reads
                # must start at a 32-aligned partition)
                va3 = vaug[tt][:].rearrange("p (h w) -> p h w", h=HL)
                nc.vector.memset(va3[:, :, 0:1], 1.0)
                nc.vector.memset(va3[:, :, 1:64], 0.0)
                nc.vector.tensor_add(
                    out=va3[:, :, 64:128],
                    in0=ps_v[:].rearrange("p (h w) -> p h w", h=HL),
                    in1=vb_bc[:].rearrange("p (h w) -> p h w", h=HL))

            def qk_chain(dc, t4, which, pool, tagname, split=False):
                """One [128, 512] QT or KT stripe chain for d-chunk dc.
                split=True runs the two 256-column halves as separate
                accumulations so the first half can start as soon as the
                xt_q0a DMA lands (startup critical path)."""
                t0 = t4 * 512
                dst, b_sb = (qt, qb_sb) if which == "q" else (kt, kb_sb)
                ps = pool.tile([128, 512], F32, tag=tagname,
                               name=f"ps{which}{dc}_{t4}")
                split = split or t4 == 0   # stripe 0 spans xt_q0a/xt_q0b
                if split:
                    for h in range(2):
                        for cc in range(CCN):
                            nc.tensor.matmul(
                                ps[:, h * 256:h * 256 + 256],
                                lhsT=w_sl(which, cc, dc),
                                rhs=xt_sl(cc, t0 + h * 256, t0 + h * 256 + 256),
                                start=(cc == 0), stop=(cc == CCN - 1))
                else:
                    for cc in range(CCN):
                        nc.tensor.matmul(ps[:], lhsT=w_sl(which, cc, dc),
                                         rhs=xt_sl(cc, t0, t0 + 512),
                                         start=(cc == 0), stop=(cc == CCN - 1))
                nc.vector.tensor_scalar_add(out=dst[dc][:, t0:t0 + 512],
                                            in0=ps[:],
                                            scalar1=b_sb[:, dc:dc + 1])

            ydma = {"i": 0}

            def proj_chain(tt, nh, pool, tagname, engs=(nc.sync,),
                           copy_eng=None):
                """Half of the output projection for t-tile tt."""
                tsl = slice(tt * 128, (tt + 1) * 128)
                nsl = slice(nh * 512, (nh + 1) * 512)
                ps_y = pool.tile([128, 512], F32, tag=tagname,
                                 name=f"psy{tt}_{nh}")
                for dc in range(DCN):
                    nc.tensor.matmul(ps_y[:], lhsT=otn[dc][:, tsl],
                                     rhs=w2_sb[:, dc * 1024 + nsl.start:
                                               dc * 1024 + nsl.stop],
                                     start=(dc == 0), stop=(dc == DCN - 1))
                y = yp.tile([128, 512], BF16, tag="y", name=f"y{tt}_{nh}")
                if copy_eng is nc.scalar:
                    nc.scalar.copy(out=y[:], in_=ps_y[:])
                else:
                    nc.vector.tensor_copy(out=y[:], in_=ps_y[:])
                eng = engs[ydma["i"] % len(engs)]
                ydma["i"] += 1
                eng.dma_start(out=out_ap[tsl, nsl], in_=y[:])

            # ---- deadline-paced filler stream ----
            st = {"jt": 0, "fillers": [], "last_pop": -10}
            LOOKAHEAD = 5

            def pace_at(t):
                # early j-tiles are deadline-driven (pair 0 deps); the
                # late region is exp-bound with PE slack every j-tile
                return 3 if t < 40 else (2 if t < 80 else 1)

            def add_filler(deadline, fn):
                st["fillers"].append((deadline, fn))
                st["fillers"].sort(key=lambda x: x[0])

            def on_jtile():
                t = st["jt"]
                st["jt"] += 1
                f = st["fillers"]
                if f and f[0][0] <= t + LOOKAHEAD:
                    f.pop(0)[1]()
                    st["last_pop"] = t
                    if f and f[0][0] <= t:      # at most one overdue extra
                        f.pop(0)[1]()
                elif len(f) > 3 and t - st["last_pop"] >= pace_at(t):
                    # keep a 3-chain reserve to cover the final
                    # drain+normalize latency after the last attention
                    f.pop(0)[1]()
                    st["last_pop"] = t

            # OT-drain + normalize of each block are DEFERRED into the
            # next block, emitted right after its first scores pair, so
            # ScalarE's exp stream never waits on the trailing drain
            pending = {"fns": []}

            def flush_pending():
                for fn in pending["fns"]:
                    fn()
                pending["fns"] = []

            def attn_pair(hp, ib):
                """Causal attention for heads (2*hp, 2*hp+1), i-block ib."""
                dc = hp
                i0 = ib * 512
                njt = 4 * ib + 4
                ots = [op.tile([128, 512], F32, tag="ot",
                               name=f"ot{hp}_{ib}_{hh}")
                       for hh in range(2)]
                # O^T matmuls lag the scores by 4-5 j-tiles; j-tiles are
                # processed in PAIRS (scores t, t+1 back to back, then two
                # lagged OT pairs) to halve the scores<->OT stationary
                # transitions, whose drain exposure costs ~100ns each
                ot_queue = []

                def do_scores(jt):
                    j0 = jt * 128
                    lo = max(0, j0 - i0)
                    stps = bp.tile([128, 1024], F32, tag="big",
                                   name=f"st{hp}_{ib}_{jt}")
                    st3 = stps[:].rearrange("p (h w) -> p h w", h=2)
                    # adjacent row-tiled pair: even head rows 0-63, odd
                    # head rows 64-127 of the kt/qt stripes
                    for hh in range(2):
                        ro = 64 * hh
                        nc.tensor.matmul(
                            st3[:, hh, lo:512],
                            lhsT=kt[dc][ro:ro + 64, j0:j0 + 128],
                            rhs=qt[dc][ro:ro + 64, i0 + lo:i0 + 512],
                            start=True, stop=True)
                    p = wp.tile([128, 1024], BF16, tag="p", bufs=7,
                                name=f"p{hp}_{ib}_{jt}")
                    p3 = p[:].rearrange("p (h w) -> p h w", h=2)
                    nc.scalar.activation(out=p3[:, :, lo:512],
                                         in_=st3[:, :, lo:512],
                                         func=Act.Exp, scale=0.125)
                    if j0 >= i0:
                        # one 3D-AP multiply masks the diagonal block of
                        # both heads (mask duplicated side by side in m0)
                        nc.vector.tensor_mul(
                            out=p3[:, :, lo:lo + 128],
                            in0=p3[:, :, lo:lo + 128],
                            in1=m0_sb[:].rearrange("p (h w) -> p h w", h=2))

                    def emit_ot(jt=jt, lo=lo, p3=p3):
                        va = vaug[jt][:].rearrange("p (h w) -> p h w", h=HL)
                        for hh in range(2):
                            nc.tensor.matmul(ots[hh][:, lo:512],
                                             lhsT=va[:, 2 * hp + hh, :],
                                             rhs=p3[:, hh, lo:512],
                                             start=(jt == 0),
                                             stop=(jt == njt - 1))

                    ot_queue.append(emit_ot)

                for jt2 in range(0, njt, 2):
                    do_scores(jt2)
                    do_scores(jt2 + 1)
                    if jt2 == 0:
                        flush_pending()
                    for _ in range(2):
                        if len(ot_queue) > 4:
                            ot_queue.pop(0)()
                        on_jtile()

                def drain_and_norm():
                    for emit in ot_queue:
                        emit()
                    # normalize by the ones-column sums (PSUM row 0 of
                    # each ot tile; O^T data in rows 64-127). Per-head
                    # staggered chains: both recips first, then the
                    # broadcasts, so the DVE never idles on gpsimd.
                    rcs = [wp.tile([1, 512], F32, tag=f"rc{hh}", bufs=2,
                                   name=f"rc{hp}_{ib}_{hh}")
                           for hh in range(2)]
                    for hh in range(2):
                        nc.vector.reciprocal_approx_fast(out=rcs[hh][:],
                                                         in_=ots[hh][0:1, :])
                    bcs = [wp.tile([64, 512], F32, tag=f"bc{hh}", bufs=2,
                                   name=f"bc{hp}_{ib}_{hh}")
                           for hh in range(2)]
                    for hh in range(2):
                        nc.gpsimd.partition_broadcast(bcs[hh][:], rcs[hh][:])
                    for hh in range(2):
                        ro = 64 * hh
                        nc.vector.tensor_mul(
                            out=otn[dc][ro:ro + 64, i0:i0 + 512],
                            in0=ots[hh][64:128, :],
                            in1=bcs[hh][:])

                pending["fns"].append(drain_and_norm)

            # ---- emission schedule ----
            # upfront: just the two QK chains pair-0 i-block-0's scores
            # need; V tiles 0-3 ride as immediate fillers (their O^T
            # consumers are the lag-deferred drain, several j-tiles out),
            # so the exp stream starts ~12us earlier
            qk_chain(0, 0, "q", bp, "big", split=True)
            qk_chain(0, 0, "k", bp, "big", split=True)

            # dense fillers with deadlines (first consuming global j-tile).
            # Global j-tile order: pair0 ib0-3 (jt 0-39), pair1 ib0-3
            # (40-79), then pairs 2/3 interleaved by i-block (see PAIR23).
            PAIR23 = [(2, 0), (3, 0), (2, 1), (3, 1), (2, 2), (3, 2),
                      (2, 3), (3, 3)]
            start_jt = {}
            tcur = 80
            for hp, ib in PAIR23:
                start_jt[(hp, ib)] = tcur
                tcur += 4 * ib + 4

            for k in range(4):
                add_filler(k - 4, lambda tt=k: v_tile(tt, fp, "fill"))
            add_filler(3, lambda: qk_chain(0, 1, "q", fp, "fill"))
            add_filler(4, lambda: qk_chain(0, 1, "k", fp, "fill"))
            add_filler(7, lambda: v_tile(4, fp, "fill"))
            add_filler(8, lambda: v_tile(5, fp, "fill"))
            for k in range(6, NT):
                ib = (k + 4) // 4 - 1  # first pair-0 i-block using it
                first_use = [0, 4, 12, 24][ib] + k
                add_filler(first_use, lambda tt=k: v_tile(tt, fp, "fill"))
            for i, t4 in enumerate((2, 3)):
                for j, w in enumerate(("q", "k")):
                    add_filler(12 * (t4 - 1) + j - 2,
                               lambda t4=t4, w=w: qk_chain(0, t4, w, fp,
                                                           "fill"))
            qk_dl1 = {0: 40, 1: 44, 2: 52, 3: 64}
            for t4 in range(4):
                for j, w in enumerate(("q", "k")):
                    add_filler(qk_dl1[t4] + j,
                               lambda t4=t4, w=w: qk_chain(1, t4, w, fp,
                                                           "fill"))
            for hp in (2, 3):
                for t4 in range(4):
                    dl = start_jt[(hp, t4)]
                    for j, w in enumerate(("q", "k")):
                        add_filler(dl + j,
                                   lambda hp=hp, t4=t4, w=w:
                                   qk_chain(hp, t4, w, fp, "fill"))

            for ib in range(IBN):
                attn_pair(0, ib)
            for ib in range(IBN):
                attn_pair(1, ib)
            for hp, ib in PAIR23:
                attn_pair(hp, ib)
                if hp == 3 and ib < IBN - 1:
                    base = st["jt"] + 2
                    for idx, (tt, nh) in enumerate(
                            (tt, nh) for tt in range(4 * ib, 4 * ib + 4)
                            for nh in range(2)):
                        # last chains of the last fillable i-block form
                        # the reserve emitted after the final drain
                        dl = (10 ** 6 if ib == IBN - 2 and idx >= 5
                              else base + 2 * idx)
                        add_filler(dl,
                                   lambda tt=tt, nh=nh:
                                   proj_chain(tt, nh, fp, "fill"))
            # drain the deferred last-block OT/normalize and leftovers,
            # then the final i-block's projection (the last exp has
            # retired: scalar's HWDGE queue is free for y and the idle
            # score-PSUM slots give a 4-deep projection pipeline)
            flush_pending()
            while st["fillers"]:
                st["fillers"].pop(0)[1]()
            for i, (tt, nh) in enumerate(
                    (tt, nh) for tt in range(4 * (IBN - 1), 4 * IBN)
                    for nh in range(2)):
                pool, tag = ((fp, "fill"), (bp, "big"))[i % 2]
                proj_chain(tt, nh, pool, tag,
                           engs=(nc.sync, nc.scalar, nc.gpsimd),
                           copy_eng=(nc.scalar if i % 2 else nc.vector))

            if DEBUG_DUMPS:
                nc.sync.dma_start(out=dbg["qt0"], in_=qt[0][:])
                nc.sync.dma_start(out=dbg["otn0"], in_=otn[0][:])

    nc.compile()
    _cached_nc = nc
    return nc


def _chunk_cols(a, cols):
    """[C_rows, W] -> [128, (C_rows/128) * len(cols)] chunk-major tiling."""
    n = a.shape[0] // 128
    return np.ascontiguousarray(
        a[:, cols].reshape(n, 128, len(range(*cols.indices(a.shape[1]))))
        .transpose(1, 0, 2).reshape(128, -1))


def _shard_inputs(x, qkv_w, qkv_b, proj_w, proj_b):
    m0 = np.triu(np.ones((128, 128), dtype=np.float32)).astype(BF16NP)
    m02 = np.ascontiguousarray(np.concatenate([m0, m0], axis=1))
    in_maps = []
    for core in range(N_CORES):
        b, g = core // 2, core % 2
        gsl = slice(g * DL, (g + 1) * DL)
        xt = x[b].T.astype(BF16NP)                        # [C, T]
        wq = qkv_w[:, gsl].astype(BF16NP)
        wk = qkv_w[:, C + g * DL:C + (g + 1) * DL].astype(BF16NP)
        wv = qkv_w[:, 2 * C + g * DL:2 * C + (g + 1) * DL].astype(BF16NP)
        w2 = proj_w[gsl, :].astype(BF16NP)
        in_maps.append({
            "xtq0a": _chunk_cols(xt, slice(0, 256)),
            "xtq0b": _chunk_cols(xt, slice(256, 512)),
            "xtq1": _chunk_cols(xt, slice(512, 1024)),
            "xthi": _chunk_cols(xt, slice(1024, 2048)),
            "wqd0": _chunk_cols(wq, slice(0, 128)),
            "wqre": _chunk_cols(wq, slice(128, DL)),
            "wkd0": _chunk_cols(wk, slice(0, 128)),
            "wkre": _chunk_cols(wk, slice(128, DL)),
            "wv": _chunk_cols(wv, slice(0, DL)),
            "w2": _chunk_cols(w2, slice(0, C)),
            "qb": np.ascontiguousarray(qkv_b[gsl].astype(np.float32)),
            "kb": np.ascontiguousarray(qkv_b[C + g * DL:C + (g + 1) * DL]
                                       .astype(np.float32)),
            "vb": np.ascontiguousarray(qkv_b[2 * C + g * DL:2 * C + (g + 1) * DL]
                                       .astype(np.float32)).reshape(1, DL),
            "m0": m02,
        })
    return in_maps


def _run(inputs, trace=False):
    x = np.asarray(inputs["x"], dtype=np.float32)
    qkv_w = np.asarray(inputs["qkv_w"], dtype=np.float32)
    qkv_b = np.asarray(inputs["qkv_b"], dtype=np.float32)
    proj_w = np.asarray(inputs["proj_w"], dtype=np.float32)
    proj_b = np.asarray(inputs["proj_b"], dtype=np.float32)

    nc = _build()
    in_maps = _shard_inputs(x, qkv_w, qkv_b, proj_w, proj_b)
    try:
        res = run_bass_kernel_spmd(nc, in_maps, core_ids=list(range(N_CORES)),
                                   trace=trace)
    except Exception:
        # transient NRT_EXEC_UNIT_UNRECOVERABLE has been observed on a
        # wedged device; one retry clears it
        import time
        time.sleep(5)
        res = run_bass_kernel_spmd(nc, in_maps, core_ids=list(range(N_CORES)),
                                   trace=trace)
    out = np.empty((B, T, C), dtype=np.float32)
    for b in range(B):
        out[b] = (res.results[2 * b]["out"].astype(np.float32)
                  + res.results[2 * b + 1]["out"].astype(np.float32)
                  + proj_b[None, :])
    return out, res.exec_time_ns


def kernel(**inputs) -> np.ndarray:
    return _run(inputs, trace=False)[0]

